# revision 1
# baseline (speedup 1.0000x reference)
"""Trainium2 Bass kernel for ClassFeatureMemoryBank proto-contrastive loss (v2).

Computes: mean over N=1M rows of  logsumexp(f_hat @ P.T / T) - (f_hat @ P.T / T)[label]
where f_hat = f / max(||f||, eps), P = [150, 128] L2-normalized prototypes.

v2 strategy (data-parallel over 8 cores, ~125k rows each):
  - Host prep: shard features, pre-cast to bf16 and pre-transpose into
    [n_groups, 128(d), 512(r)] blocks -> single HWDGE DMA per group with
    1KB contiguous partition lines; no on-device transposes or cast-DMAs.
  - DVE: f2 = fT*fT (one op per group)
  - PE: q[r] = ones-matmul over f2 (4 tiny matmuls per group, row-major out)
        logits[r, c] = fT.T @ protosT (4 matmuls per group)
  - DVE: Newton rsqrt (bit-hack + 2 iters) on q batched over 8 groups,
        with the 1/TEMP factor folded into the last iteration
  - ACT: expb = Exp(logits * s/T) with per-partition scale AND fused
        accum_out -> sumexp column (one op per 128-row tile)
  - GPSIMD: psc = (iota == label) * expb, accum -> mexp column
  - Host: loss = mean(log(sumexp) - log(mexp)) in float64
"""
import sys

sys.path.insert(0, "/opt/trn_rl_repo")

import numpy as np
import ml_dtypes
from contextlib import ExitStack

import concourse.bass as bass
import concourse.tile as tile
from concourse import bacc, mybir
from concourse.bass_utils import run_bass_kernel_spmd

F32 = mybir.dt.float32
BF16 = mybir.dt.bfloat16
I32 = mybir.dt.int32
ALU = mybir.AluOpType

N_CORES = 8
D = 128
C = 150
TEMP = 0.15
GT = 4                 # tiles per group (PSUM-sized)
GW = GT * 128          # rows per group = 512
NB = 8                 # groups per newton batch

N_FULL = 1_000_000
ROWS_PER_CORE_FULL = 125_952   # = 512 * 246; 8*this >= 1M
G = GT                 # kept for test.py compat (rows_per_core % (128*G) == 0)

_NC_CACHE = {}


def build_nc(rows_per_core: int, nb=NB, qcopy_dve=True, sumexp_dve=False):
    assert rows_per_core % GW == 0
    n_groups = rows_per_core // GW
    n_tiles = rows_per_core // 128

    nc = bacc.Bacc("TRN2", target_bir_lowering=False, debug=False)
    fT = nc.dram_tensor("fT", [n_groups, 128, GW], BF16,
                        kind="ExternalInput").ap()
    labelsf = nc.dram_tensor("labelsf", [128, n_tiles], F32,
                             kind="ExternalInput").ap()
    protosT = nc.dram_tensor("protosT", [128, C], BF16,
                             kind="ExternalInput").ap()
    iota = nc.dram_tensor("iota", [128, C], BF16, kind="ExternalInput").ap()

    out_sumexp = nc.dram_tensor("sumexp", [128, n_tiles], F32,
                                kind="ExternalOutput").ap()
    out_mexp = nc.dram_tensor("mexp", [128, n_tiles], F32,
                              kind="ExternalOutput").ap()

    with tile.TileContext(nc) as tc, ExitStack() as ctx:
        const = ctx.enter_context(tc.tile_pool(name="const", bufs=1))
        fpool = ctx.enter_context(tc.tile_pool(name="f", bufs=nb + 3))
        f2pool = ctx.enter_context(tc.tile_pool(name="f2", bufs=3))
        epool = ctx.enter_context(tc.tile_pool(name="expb", bufs=3))
        spool = ctx.enter_context(tc.tile_pool(name="s", bufs=2))
        scpool = ctx.enter_context(tc.tile_pool(name="scratch", bufs=2))
        qpsum = ctx.enter_context(tc.tile_pool(name="qp", bufs=2, space="PSUM"))
        lpsum = ctx.enter_context(tc.tile_pool(name="lp", bufs=3, space="PSUM"))

        protosT_sb = const.tile([128, C], BF16)
        nc.sync.dma_start(protosT_sb[:], protosT[:, :])
        iota_sb = const.tile([128, C], BF16)
        nc.sync.dma_start(iota_sb[:], iota[:, :])
        labelsf_sb = const.tile([128, n_tiles], F32)
        nc.sync.dma_start(labelsf_sb[:], labelsf[:, :])
        ones_sb = const.tile([128, 1], BF16)
        nc.vector.memset(ones_sb[:], 1.0)
        kmagic = const.tile([128, GT * nb], I32)
        nc.vector.memset(kmagic[:], 0x5F3759DF)

        sumexp_buf = const.tile([128, n_tiles], F32)
        mexp_buf = const.tile([128, n_tiles], F32)

        def newton_rsqrt_scaled(sT_t, q_ps, w):
            """sT = (1/sqrt(q)) / TEMP, elementwise on [128, w]. q in PSUM."""
            q_sb = scpool.tile([128, GT * nb], F32, tag="nt_q")
            if qcopy_dve:
                nc.vector.tensor_scalar(q_sb[:, 0:w], q_ps[:, 0:w], 1.0, None,
                                        ALU.mult)
            else:
                nc.scalar.copy(q_sb[:, 0:w], q_ps[:, 0:w])
            bs = scpool.tile([128, GT * nb], I32, tag="nt_bs")
            nc.vector.tensor_scalar(bs[:, 0:w], q_sb[:, 0:w].bitcast(I32), 1,
                                    None, ALU.logical_shift_right)
            y0 = scpool.tile([128, GT * nb], I32, tag="nt_y0")
            nc.vector.tensor_tensor(y0[:, 0:w], kmagic[:, 0:w], bs[:, 0:w],
                                    ALU.subtract)
            y0f = y0[:, 0:w].bitcast(F32)
            t = scpool.tile([128, GT * nb], F32, tag="nt_t")
            # iter 1:  t = (q * -0.5) * y;  t = t * y;  y1 = (t + 1.5) * y
            y1 = scpool.tile([128, GT * nb], F32, tag="nt_y1")
            nc.vector.scalar_tensor_tensor(t[:, 0:w], q_sb[:, 0:w], -0.5, y0f,
                                           ALU.mult, ALU.mult)
            nc.vector.tensor_tensor(t[:, 0:w], t[:, 0:w], y0f, ALU.mult)
            nc.vector.scalar_tensor_tensor(y1[:, 0:w], t[:, 0:w], 1.5, y0f,
                                           ALU.add, ALU.mult)
            # iter 2 with 1/TEMP folded:
            #   t = (q * -0.5) * y1           [= -q*y1/2]
            #   t = (t * 1/T) * y1            [= -q*y1^2/(2T)]
            #   sT = (t + 1.5/T) * y1         [= (1.5 - q*y1^2/2) * y1 / T]
            nc.vector.scalar_tensor_tensor(t[:, 0:w], q_sb[:, 0:w], -0.5,
                                           y1[:, 0:w], ALU.mult, ALU.mult)
            nc.vector.scalar_tensor_tensor(t[:, 0:w], t[:, 0:w], 1.0 / TEMP,
                                           y1[:, 0:w], ALU.mult, ALU.mult)
            nc.vector.scalar_tensor_tensor(sT_t[:, 0:w], t[:, 0:w],
                                           1.5 / TEMP, y1[:, 0:w],
                                           ALU.add, ALU.mult)

        n_batches = (n_groups + nb - 1) // nb
        for b in range(n_batches):
            gis = list(range(b * nb, min((b + 1) * nb, n_groups)))
            w = len(gis) * GT

            # phase A: load + f2 + q matmuls
            q_ps = qpsum.tile([128, GT * nb], F32)
            f_tiles = {}
            for j, gi in enumerate(gis):
                ft = fpool.tile([128, GT, D], BF16)
                nc.sync.dma_start(ft[:], fT[gi].rearrange("p (g r) -> p g r",
                                                          g=GT))
                f_tiles[gi] = ft
                f2 = f2pool.tile([128, GT, D], BF16)
                nc.gpsimd.tensor_tensor(f2[:], ft[:], ft[:], ALU.mult)
                for g in range(GT):
                    col = j * GT + g
                    nc.tensor.matmul(q_ps[:, col:col + 1], f2[:, g, :],
                                     ones_sb[:], start=True, stop=True)

            sT = spool.tile([128, GT * nb], F32)
            newton_rsqrt_scaled(sT, q_ps, w)

            # phase B: logits matmul -> exp(scale)+sumexp -> label pick
            psc = scpool.tile([128, C], BF16, tag="psc")
            for j, gi in enumerate(gis):
                ft = f_tiles[gi]
                logits = lpsum.tile([128, GT, 256], F32)
                for g in range(GT):
                    nc.tensor.matmul(logits[:, g, 0:C], ft[:, g, :],
                                     protosT_sb[:], start=True, stop=True)
                expb = epool.tile([128, GT, C], BF16)
                ti0 = gi * GT
                for g in range(GT):
                    ti = gi * GT + g
                    col = j * GT + g
                    if sumexp_dve:
                        nc.scalar.activation(expb[:, g, :], logits[:, g, 0:C],
                                             mybir.ActivationFunctionType.Exp,
                                             bias=0.0,
                                             scale=sT[:, col:col + 1])
                    else:
                        nc.scalar.activation(expb[:, g, :], logits[:, g, 0:C],
                                             mybir.ActivationFunctionType.Exp,
                                             bias=0.0, scale=sT[:, col:col + 1],
                                             accum_out=sumexp_buf[:, ti:ti + 1])
                    nc.vector.scalar_tensor_tensor(
                        psc[:], iota_sb[:], labelsf_sb[:, ti:ti + 1],
                        expb[:, g, :], ALU.is_equal, ALU.mult,
                        mexp_buf[:, ti:ti + 1])
                if sumexp_dve:
                    nc.vector.tensor_reduce(
                        sumexp_buf[:, ti0:ti0 + GT], expb[:, :, :],
                        mybir.AxisListType.X, ALU.add)

        nc.sync.dma_start(out_sumexp[:, :], sumexp_buf[:])
        nc.sync.dma_start(out_mexp[:, :], mexp_buf[:])

    nc.compile()
    return nc


def _get_nc(rows_per_core):
    if rows_per_core not in _NC_CACHE:
        _NC_CACHE[rows_per_core] = build_nc(rows_per_core)
    return _NC_CACHE[rows_per_core]


def _prep_core_inputs(features, labels, prototypes, rows_per_core):
    """Shard + host-side layout prep. Returns (in_maps, n_valid_per_core)."""
    n = features.shape[0]
    n_tiles = rows_per_core // 128
    n_groups = rows_per_core // GW

    protosT_np = np.ascontiguousarray(
        prototypes.T.astype(np.float32)).astype(ml_dtypes.bfloat16)
    iota_np = np.ascontiguousarray(
        np.broadcast_to(np.arange(C, dtype=np.float32), (128, C))
    ).astype(ml_dtypes.bfloat16)

    in_maps = []
    n_valid = []
    for c in range(N_CORES):
        lo = c * rows_per_core
        hi = min(n, lo + rows_per_core)
        valid = max(0, hi - lo)
        n_valid.append(valid)
        if valid == rows_per_core:
            fshard = features[lo:hi]
            lshard = labels[lo:hi]
        else:
            fshard = np.zeros((rows_per_core, D), dtype=np.float32)
            fshard[:, 0] = 1.0  # unit rows: q=1, harmless
            lshard = np.zeros(rows_per_core, dtype=np.int64)
            if valid > 0:
                fshard[:valid] = features[lo:hi]
                lshard[:valid] = labels[lo:hi]
        # fT[gi, d, r] = f[gi*GW + r, d], bf16
        fT = np.ascontiguousarray(
            fshard.reshape(n_groups, GW, D).transpose(0, 2, 1)
        ).astype(ml_dtypes.bfloat16)
        # labelsf[p, t] = label of row t*128 + p
        labelsf = np.ascontiguousarray(
            lshard.reshape(n_tiles, 128).T).astype(np.float32)
        in_maps.append({
            "fT": fT,
            "labelsf": labelsf,
            "protosT": protosT_np,
            "iota": iota_np,
        })
    return in_maps, n_valid


def run_cores(features, labels, prototypes, rows_per_core, trace=False):
    nc = _get_nc(rows_per_core)
    in_maps, n_valid = _prep_core_inputs(features, labels, prototypes,
                                         rows_per_core)
    res = run_bass_kernel_spmd(nc, in_maps, core_ids=list(range(N_CORES)),
                               trace=trace)
    return res, n_valid


def _reduce_host(res, n_valid, rows_per_core, n_total):
    n_tiles = rows_per_core // 128
    total = 0.0
    for c in range(N_CORES):
        valid = n_valid[c]
        if valid == 0:
            continue
        sumexp = res.results[c]["sumexp"].astype(np.float64)  # [128, n_tiles]
        mexp = res.results[c]["mexp"].astype(np.float64)
        # row index of (p, t) is t*128 + p
        p = np.arange(128)[:, None]
        t = np.arange(n_tiles)[None, :]
        mask = (t * 128 + p) < valid
        logz = np.log(sumexp[mask])
        picked = np.log(mexp[mask])
        total += (logz - picked).sum()
    return np.float32(total / n_total)


def kernel(features, labels, prototypes):
    features = np.asarray(features, dtype=np.float32)
    labels = np.asarray(labels)
    prototypes = np.asarray(prototypes, dtype=np.float32)
    n = features.shape[0]
    if n == N_FULL:
        rows_per_core = ROWS_PER_CORE_FULL
    else:
        # smallest multiple of GW covering n/8
        per = (n + N_CORES - 1) // N_CORES
        rows_per_core = ((per + GW - 1) // GW) * GW
    res, n_valid = run_cores(features, labels, prototypes, rows_per_core)
    return _reduce_host(res, n_valid, rows_per_core, n)


if __name__ == "__main__":
    # quick self-test with small n
    rng = np.random.default_rng(0)
    n = 8 * GW * 17 + 300   # exercises padding path too
    f = rng.normal(size=(n, D)).astype(np.float32)
    lab = rng.integers(0, C, size=n).astype(np.int64)
    p = rng.normal(size=(C, D)).astype(np.float32)
    p /= np.linalg.norm(p, axis=1, keepdims=True)
    got = kernel(f, lab, p)

    fh = f / np.maximum(np.linalg.norm(f, axis=1, keepdims=True), 1e-12)
    logits = fh @ p.T / TEMP
    m = logits.max(axis=1, keepdims=True)
    logz = np.log(np.exp(logits - m).sum(1)) + m[:, 0]
    picked = np.take_along_axis(logits, lab[:, None], axis=1)[:, 0]
    want = (logz - picked).mean()
    print("got:", got, "want:", want, "rel:", abs(got / want - 1))



# revision 2
# speedup vs baseline: 1.5108x; 1.5108x over previous
"""Trainium2 Bass kernel for ClassFeatureMemoryBank proto-contrastive loss (v7).

loss = mean_r [ logsumexp_c(f_hat_r . p_c / T) - (f_hat_r . p_label_r / T) ]

v7 strategy (vs v2 baseline): eliminate ALL per-tile ACT/DVE instructions.
  - Host: sort rows by label, pad each class to 128-row tiles -> every tile
    is single-label. Ship features ROW-major (fR), tile-blocked.
  - ACT (batched): f2 = Square(fR); expb = Exp(logits) with constant scale.
  - GPSIMD: first tree level of both reductions (sum over d; sum over c).
  - DVE (batched): rest of q-tree, Newton rsqrt (s=1/(T*||f||)) on [128,nt]
    layout, f_hat = fR * s (free-dim broadcast), sumexp tree.
  - DMA xbar: blocked transpose f_hatR -> f_hatT per 18-tile chunk.
  - PE: per tile ONE stationary f_hatT, two matmuls: logits (protosT,
    padded to 160 cols with zeros -> sumexp gets +10, host subtracts) and
    picked (gT2 = per-tile label prototype column, FD=1).
  - Host: loss = mean(log(sumexp-10) - picked) over valid rows.
"""
import sys

sys.path.insert(0, "/opt/trn_rl_repo")

import numpy as np
import ml_dtypes
from contextlib import ExitStack

import concourse.bass as bass
import concourse.tile as tile
from concourse import bacc, mybir
from concourse.bass_utils import run_bass_kernel_spmd

F32 = mybir.dt.float32
BF16 = mybir.dt.bfloat16
I32 = mybir.dt.int32
ALU = mybir.AluOpType
AX = mybir.AxisListType

N_CORES = 8
D = 128
C = 150
CP = 160              # padded class columns (150 real + 10 zero)
SLOT = 170            # psum slot stride (3 slots * 170 * 4B = 2040 <= bank)
TEMP = 0.15
CH_T = 18             # tiles per chunk
EB = 9                # tiles per exp batch (2 per chunk)
NEWT_CH = 8           # chunks per newton batch

N_FULL = 1_000_000

_NC_CACHE = {}


def build_nc(nt: int):
    """nt tiles of 128 rows per core; nt % CH_T == 0."""
    assert nt % CH_T == 0
    nch = nt // CH_T
    half = (nt + 1) // 2
    assert half <= 512

    nc = bacc.Bacc("TRN2", target_bir_lowering=False, debug=False)
    fR = nc.dram_tensor("fR", [nch, 128, CH_T, D], BF16,
                        kind="ExternalInput").ap()
    gT2 = nc.dram_tensor("gT2", [128, nt], BF16, kind="ExternalInput").ap()
    protosT = nc.dram_tensor("protosT", [128, CP], BF16,
                             kind="ExternalInput").ap()
    out_sumexp = nc.dram_tensor("sumexp", [128, nt], F32,
                                kind="ExternalOutput").ap()
    out_picked = nc.dram_tensor("picked", [128, nt], F32,
                                kind="ExternalOutput").ap()

    NEWT_COLS = NEWT_CH * CH_T

    with tile.TileContext(nc) as tc, ExitStack() as ctx:
        const = ctx.enter_context(tc.tile_pool(name="const", bufs=1))
        frpool = ctx.enter_context(tc.tile_pool(name="fr", bufs=NEWT_CH + 3))
        f2pool = ctx.enter_context(tc.tile_pool(name="f2", bufs=3))
        fhpool = ctx.enter_context(tc.tile_pool(name="fh", bufs=3))
        ftpool = ctx.enter_context(tc.tile_pool(name="ft", bufs=3))
        ebpool = ctx.enter_context(tc.tile_pool(name="eb", bufs=4))
        v1pool = ctx.enter_context(tc.tile_pool(name="v1", bufs=2))
        v2pool = ctx.enter_context(tc.tile_pool(name="v2", bufs=2))
        v3pool = ctx.enter_context(tc.tile_pool(name="v3", bufs=2))
        u1pool = ctx.enter_context(tc.tile_pool(name="u1", bufs=2))
        u2pool = ctx.enter_context(tc.tile_pool(name="u2", bufs=2))
        u3pool = ctx.enter_context(tc.tile_pool(name="u3", bufs=2))
        ntpool = ctx.enter_context(tc.tile_pool(name="nt", bufs=2))
        lpool = ctx.enter_context(tc.tile_pool(name="lp", bufs=2,
                                               space="PSUM"))
        ppool = ctx.enter_context(tc.tile_pool(name="pp", bufs=1,
                                               space="PSUM"))

        protosT_sb = const.tile([128, CP], BF16)
        nc.sync.dma_start(protosT_sb[:], protosT[:, :])
        gT2_sb = const.tile([128, nt], BF16)
        nc.sync.dma_start(gT2_sb[:], gT2[:, :])
        kmagic = const.tile([128, NEWT_COLS], I32)
        nc.vector.memset(kmagic[:], 0x5F3759DF)

        q_buf = const.tile([128, nt], F32)
        sT_buf = const.tile([128, nt], BF16)
        sum_buf = const.tile([128, nt], F32)

        picked_ps = [ppool.tile([128, half], F32, name=f"picked{i}")
                     for i in range(2)]

        def newton_rsqrt_scaled(dst_bf16, q_ap, w):
            """dst = (1/sqrt(q)) / TEMP elementwise on [128, w] f32 SBUF."""
            bs = ntpool.tile([128, NEWT_COLS], I32, tag="nt_bs")
            nc.vector.tensor_scalar(bs[:, 0:w], q_ap.bitcast(I32), 1,
                                    None, ALU.logical_shift_right)
            y0 = ntpool.tile([128, NEWT_COLS], I32, tag="nt_y0")
            nc.vector.tensor_tensor(y0[:, 0:w], kmagic[:, 0:w], bs[:, 0:w],
                                    ALU.subtract)
            y0f = y0[:, 0:w].bitcast(F32)
            t = ntpool.tile([128, NEWT_COLS], F32, tag="nt_t")
            y1 = ntpool.tile([128, NEWT_COLS], F32, tag="nt_y1")
            nc.vector.scalar_tensor_tensor(t[:, 0:w], q_ap, -0.5, y0f,
                                           ALU.mult, ALU.mult)
            nc.vector.tensor_tensor(t[:, 0:w], t[:, 0:w], y0f, ALU.mult)
            nc.vector.scalar_tensor_tensor(y1[:, 0:w], t[:, 0:w], 1.5, y0f,
                                           ALU.add, ALU.mult)
            nc.vector.scalar_tensor_tensor(t[:, 0:w], q_ap, -0.5,
                                           y1[:, 0:w], ALU.mult, ALU.mult)
            nc.vector.scalar_tensor_tensor(t[:, 0:w], t[:, 0:w], 1.0 / TEMP,
                                           y1[:, 0:w], ALU.mult, ALU.mult)
            nc.vector.scalar_tensor_tensor(dst_bf16, t[:, 0:w],
                                           1.5 / TEMP, y1[:, 0:w],
                                           ALU.add, ALU.mult)

        fr_tiles = {}
        n_batches = (nch + NEWT_CH - 1) // NEWT_CH

        for b in range(n_batches):
            cs = list(range(b * NEWT_CH, min((b + 1) * NEWT_CH, nch)))

            # phase A: load + square + q-tree per chunk
            for c in cs:
                fr = frpool.tile([128, CH_T, D], BF16)
                nc.sync.dma_start(fr[:], fR[c])
                fr_tiles[c] = fr
                f2 = f2pool.tile([128, CH_T, D], BF16)
                nc.scalar.activation(f2[:], fr[:],
                                     mybir.ActivationFunctionType.Square)
                v1 = v1pool.tile([128, CH_T, 64], BF16)
                nc.gpsimd.tensor_tensor(v1[:], f2[:, :, 0:64],
                                        f2[:, :, 64:128], ALU.add)
                v2 = v2pool.tile([128, CH_T, 32], BF16)
                nc.vector.tensor_tensor(v2[:], v1[:, :, 0:32],
                                        v1[:, :, 32:64], ALU.add)
                v3 = v3pool.tile([128, CH_T, 16], BF16)
                nc.vector.tensor_tensor(v3[:], v2[:, :, 0:16],
                                        v2[:, :, 16:32], ALU.add)
                nc.vector.tensor_reduce(
                    q_buf[:, c * CH_T:(c + 1) * CH_T], v3[:], AX.X, ALU.add)

            # newton for the batch
            lo = cs[0] * CH_T
            hi = (cs[-1] + 1) * CH_T
            newton_rsqrt_scaled(sT_buf[:, lo:hi], q_buf[:, lo:hi], hi - lo)

            # phase B: normalize + transpose + matmuls + exp + sumexp
            for c in cs:
                fr = fr_tiles.pop(c)
                t0 = c * CH_T
                fh = fhpool.tile([128, CH_T, D], BF16)
                nc.vector.tensor_tensor(
                    fh[:], fr[:],
                    sT_buf[:, t0:t0 + CH_T].to_broadcast((128, CH_T, D)),
                    ALU.mult)
                ft = ftpool.tile([128, CH_T, D], BF16)
                nc.sync.dma_start_transpose(
                    ft[:], fh[:].rearrange("p t d -> p (t d)"))

                for h in range(2):  # two 9-tile exp batches per chunk
                    lp = lpool.tile([128, 3, 512], F32)
                    for j in range(EB):
                        jt = h * EB + j           # tile within chunk
                        t = t0 + jt               # global tile
                        slot = lp[:, j // 3, (j % 3) * SLOT:
                                  (j % 3) * SLOT + CP]
                        nc.tensor.matmul(slot, ft[:, jt, :],
                                         protosT_sb[:], start=True, stop=True)
                        ph, pc = (0, t) if t < half else (1, t - half)
                        nc.tensor.matmul(picked_ps[ph][:, pc:pc + 1],
                                         ft[:, jt, :], gT2_sb[:, t:t + 1],
                                         start=True, stop=True)
                    eb = ebpool.tile([128, EB, CP], BF16)
                    lp_4d = lp[:, :, 0:510].rearrange(
                        "p b (j x) -> p b j x", x=SLOT)[:, :, :, 0:CP]
                    nc.scalar.activation(
                        eb[:].rearrange("p (b j) x -> p b j x", j=3),
                        lp_4d, mybir.ActivationFunctionType.Exp,
                        bias=0.0, scale=1.0)
                    u1 = u1pool.tile([128, EB, 80], BF16)
                    nc.gpsimd.tensor_tensor(u1[:], eb[:, :, 0:80],
                                            eb[:, :, 80:160], ALU.add)
                    u2 = u2pool.tile([128, EB, 40], BF16)
                    nc.vector.tensor_tensor(u2[:], u1[:, :, 0:40],
                                            u1[:, :, 40:80], ALU.add)
                    u3 = u3pool.tile([128, EB, 20], BF16)
                    nc.vector.tensor_tensor(u3[:], u2[:, :, 0:20],
                                            u2[:, :, 20:40], ALU.add)
                    nc.vector.tensor_reduce(
                        sum_buf[:, t0 + h * EB:t0 + h * EB + EB],
                        u3[:], AX.X, ALU.add)

        nc.sync.dma_start(out_sumexp[:, :], sum_buf[:])
        picked_sb = const.tile([128, nt], F32)
        nc.vector.tensor_scalar(picked_sb[:, 0:half], picked_ps[0][:],
                                1.0, None, ALU.mult)
        nc.vector.tensor_scalar(picked_sb[:, half:nt],
                                picked_ps[1][:, 0:nt - half],
                                1.0, None, ALU.mult)
        nc.sync.dma_start(out_picked[:, :], picked_sb[:])

    nc.compile()
    return nc


def _get_nc(nt):
    if nt not in _NC_CACHE:
        _NC_CACHE[nt] = build_nc(nt)
    return _NC_CACHE[nt]


def _prep_inputs(features, labels, prototypes):
    """Sort rows by label, pad classes to tile boundaries, shard, block.

    Returns (in_maps, valid_masks [core][128, nt], nt)."""
    n = features.shape[0]
    labels = np.asarray(labels).astype(np.int64)
    order = np.argsort(labels, kind="stable")
    sorted_labels = labels[order]

    # class boundaries in sorted order
    counts = np.bincount(sorted_labels, minlength=C)
    tiles_per_class = (counts + 127) // 128          # [C]
    total_tiles = int(tiles_per_class.sum())

    # per-core tile count: multiple of CH_T covering total_tiles/8
    per = (total_tiles + N_CORES - 1) // N_CORES
    nt = ((per + CH_T - 1) // CH_T) * CH_T
    nt_total = nt * N_CORES
    assert nt_total >= total_tiles

    # row_idx[tile, j] = original row index or -1 (dummy)
    row_idx = np.full((nt_total, 128), -1, dtype=np.int64)
    tile_label = np.zeros(nt_total, dtype=np.int64)
    tpos = 0
    rpos = 0
    for ccls in range(C):
        cnt = int(counts[ccls])
        ntile = int(tiles_per_class[ccls])
        if ntile == 0:
            continue
        idx = order[rpos:rpos + cnt]
        rpos += cnt
        block = np.full(ntile * 128, -1, dtype=np.int64)
        block[:cnt] = idx
        row_idx[tpos:tpos + ntile] = block.reshape(ntile, 128)
        tile_label[tpos:tpos + ntile] = ccls
        tpos += ntile

    protos_bf = np.ascontiguousarray(prototypes).astype(np.float32)

    feats_bf = features.astype(ml_dtypes.bfloat16)
    dummy = np.zeros(D, dtype=ml_dtypes.bfloat16)
    dummy[0] = 1.0

    protosT_np = np.zeros((128, CP), dtype=ml_dtypes.bfloat16)
    protosT_np[:, 0:C] = protos_bf.T.astype(ml_dtypes.bfloat16)

    in_maps = []
    masks = []
    nch = nt // CH_T
    for core in range(N_CORES):
        ti = row_idx[core * nt:(core + 1) * nt]          # [nt, 128]
        tl = tile_label[core * nt:(core + 1) * nt]       # [nt]
        # gather rows -> [nt*128, D] bf16
        flat = ti.reshape(-1)
        fr = np.empty((nt * 128, D), dtype=ml_dtypes.bfloat16)
        valid = flat >= 0
        fr[valid] = feats_bf[flat[valid]]
        fr[~valid] = dummy
        # block: [nch, CH_T, 128, D] -> [nch, 128, CH_T, D]
        frb = np.ascontiguousarray(
            fr.reshape(nch, CH_T, 128, D).transpose(0, 2, 1, 3))
        gT2 = np.ascontiguousarray(
            protos_bf[tl].T).astype(ml_dtypes.bfloat16)   # [128, nt]
        in_maps.append({"fR": frb, "gT2": gT2, "protosT": protosT_np})
        masks.append(ti.T >= 0)                           # [128, nt]
    return in_maps, masks, nt


def _reduce_host(res, masks, n_total):
    total = 0.0
    for core in range(N_CORES):
        m = masks[core]
        if not m.any():
            continue
        sumexp = res.results[core]["sumexp"].astype(np.float64)
        picked = res.results[core]["picked"].astype(np.float64)
        se = sumexp[m] - 10.0    # remove the 10 zero-pad columns (exp(0)=1)
        total += (np.log(se) - picked[m]).sum()
    return np.float32(total / n_total)


def kernel(features, labels, prototypes):
    features = np.asarray(features, dtype=np.float32)
    labels = np.asarray(labels)
    prototypes = np.asarray(prototypes, dtype=np.float32)
    n = features.shape[0]
    in_maps, masks, nt = _prep_inputs(features, labels, prototypes)
    nc = _get_nc(nt)
    res = run_bass_kernel_spmd(nc, in_maps, core_ids=list(range(N_CORES)))
    return _reduce_host(res, masks, n)


if __name__ == "__main__":
    rng = np.random.default_rng(0)
    n = 40_000
    f = rng.normal(size=(n, D)).astype(np.float32)
    lab = rng.integers(0, C, size=n).astype(np.int64)
    p = rng.normal(size=(C, D)).astype(np.float32)
    p /= np.linalg.norm(p, axis=1, keepdims=True)
    got = kernel(f, lab, p)

    fh = f / np.maximum(np.linalg.norm(f, axis=1, keepdims=True), 1e-12)
    logits = fh @ p.T / TEMP
    m = logits.max(axis=1, keepdims=True)
    logz = np.log(np.exp(logits - m).sum(1)) + m[:, 0]
    picked = np.take_along_axis(logits, lab[:, None], axis=1)[:, 0]
    want = (logz - picked).mean()
    print("got:", got, "want:", want, "rel:", abs(got / want - 1))
